# revision 10
# baseline (speedup 1.0000x reference)
"""Multi-head attention (B=4, S=2048, D=1024, H=16) on 8 Trainium2 cores.

Sharding: core c -> head-pair p = c (2 heads, 128 output dims), all 4
batches.  Every core runs the same per-batch k-loop trip counts
kc_b = ceil(valid_len[b]/128), so the key-padding truncation is SPMD-uniform.
W_o is row-split by head-pair; each core emits a full-shape [B, S, D] fp16
partial and the host sums the 8 partials.

v7: engines execute their instruction queues in emission order, so overlap
must be programmed, not hoped for.  The kernel is one global software-
pipelined stream over attention tiles (b, qb, kc):

    scores(t) -> exp(t) -> [av/den(t-1)] -> [norm when qb done] -> fill

where "fill" pops one closure from a queue holding Q/K/V-projection matmul
groups of the next batch and O-projection chunks of finished query blocks.
This keeps the PE busy during every exp wait and keeps ACT back-to-back.

Math per tile: scores = row-tiled concurrent matmul pair (K=64 at row
groups 0/64); AV = col-tiled concurrent pair (M=64 at col groups 0/64);
denominators via a second col-tiled pair with masked-ones lhsT, landing
den on the same partitions as the AV dims -> normalization is one
lane-aligned reciprocal_approx_fast + tensor_mul from PSUM.

PSUM: sc 2x2 banks + avden 2 + pqk 1 + pv 1 = 8 banks.
"""

import contextlib

import numpy as np
import ml_dtypes

import concourse.bacc as bacc
import concourse.mybir as mybir
import concourse.tile as tile
from concourse.bass_utils import run_bass_kernel_spmd

BF16 = mybir.dt.bfloat16
F16 = mybir.dt.float16
F32 = mybir.dt.float32
AF = mybir.ActivationFunctionType

B, S, D, H, HD = 4, 2048, 1024, 16, 64

_cache = {}


class _Emitter:
    def __init__(self, nc, tc, ap, kcs):
        self.nc = nc
        self.ap = ap
        self.kcs = kcs
        self.fills = []
        self._seq = 0

        es = self.es = contextlib.ExitStack()
        const = es.enter_context(tc.tile_pool(name="const", bufs=1))
        resid = es.enter_context(tc.tile_pool(name="resid", bufs=1))
        self.stream = es.enter_context(tc.tile_pool(name="stream", bufs=2))
        self.expool = es.enter_context(tc.tile_pool(name="expool", bufs=3))
        self.wrk = es.enter_context(tc.tile_pool(name="wrk", bufs=2))
        self.psum = es.enter_context(
            tc.tile_pool(name="psum", bufs=1, space="PSUM"))

        self.wq = const.tile([128, 8, 128], BF16, tag="wq", name="wq")
        self.wk = const.tile([128, 8, 128], BF16, tag="wk", name="wk")
        self.wv = const.tile([128, 8, 2, HD], BF16, tag="wv", name="wv")
        self.wo = const.tile([128, D], BF16, tag="wo", name="wo")
        self.ones = const.tile([128, 5, 64], BF16, tag="ones", name="ones")
        for n, t in [("wq", self.wq), ("wk", self.wk), ("wv", self.wv),
                     ("wo", self.wo), ("ones", self.ones)]:
            nc.sync.dma_start(t[:], ap[n])

        self.qT = [resid.tile([128, S], BF16, tag=f"qT{b}", name=f"qT{b}")
                   for b in range(B)]
        self.kT = [resid.tile([128, kcs[b] * 128], BF16, tag=f"kT{b}",
                              name=f"kT{b}") for b in range(B)]
        self.ctx = [resid.tile([128, S], BF16, tag=f"ctx{b}", name=f"ctx{b}")
                    for b in range(B)]
        self.v = [[resid.tile([128, 2, HD], BF16, tag=f"v{b}_{i}",
                              name=f"v{b}_{i}") for i in range(kcs[b])]
                  for b in range(B)]

    # ---- fill closures (projections of a later batch, O-proj chunks) ----

    def push_proj(self, b, deadlines=False):
        """Emit input DMAs for batch b now; queue its matmul groups.

        With deadlines=True (first batch), each closure carries the index of
        the earliest attention tile of batch b that needs its output, so the
        tile loop can force-pop them just in time and start attention while
        the rest of the projections interleave."""
        nc, ap, kcb = self.nc, self.ap, self.kcs[b]
        nk = kcb * 128
        xqt, xkt, xvt = {}, {}, {}
        for hf in range(2):
            t = self.stream.tile([128, 8, 1024], BF16, tag="xq", bufs=3,
                                 name="xqt")
            nc.sync.dma_start(t[:], ap[f"xq{b}"][:, :, hf * 1024:(hf + 1) * 1024])
            xqt[hf] = t
        for hf in range((nk + 1023) // 1024):
            n = min(1024, nk - hf * 1024)
            t = self.stream.tile([128, 8, 1024], BF16, tag="xk", name="xkt")
            nc.sync.dma_start(t[:, :, 0:n],
                              ap[f"xk{b}"][:, :, hf * 1024:hf * 1024 + n])
            xkt[hf] = t
            t = self.stream.tile([128, 8, 1024], BF16, tag="xv", name="xvt")
            nc.sync.dma_start(t[:, :, 0:n],
                              ap[f"xv{b}"][:, :, hf * 1024:hf * 1024 + n])
            xvt[hf] = t

        def qgroup(hf, q2):
            def go():
                qb = hf * 2 + q2
                psq = self.psum.tile([128, 512], F32, tag="pqk", bufs=1,
                                     name="psq")
                for dj in range(8):
                    nc.tensor.matmul(psq[:], self.wq[:, dj, :],
                                     xqt[hf][:, dj, q2 * 512:(q2 + 1) * 512],
                                     start=(dj == 0), stop=(dj == 7))
                nc.vector.tensor_copy(
                    self.qT[b][:, qb * 512:(qb + 1) * 512], psq[:])
            return go

        def kgroup(hf, k2, m):
            def go():
                psk = self.psum.tile([128, 512], F32, tag="pqk", bufs=1,
                                     name="psk")
                for dj in range(8):
                    nc.tensor.matmul(psk[:, 0:m], self.wk[:, dj, :],
                                     xkt[hf][:, dj, k2 * 512:k2 * 512 + m],
                                     start=(dj == 0), stop=(dj == 7))
                o = hf * 1024 + k2 * 512
                nc.vector.tensor_copy(self.kT[b][:, o:o + m], psk[:, 0:m])
            return go

        def vgroup(hf, s2s):
            def go():
                for s2 in s2s:
                    sc = hf * 8 + s2
                    psv = self.psum.tile([128, 2, HD], F32, tag="pv", bufs=1,
                                         name="psv")
                    for dj in range(8):
                        nc.tensor.matmul(
                            psv[:], xvt[hf][:, dj, s2 * 128:(s2 + 1) * 128],
                            self.wv[:, dj, :, :],
                            start=(dj == 0), stop=(dj == 7))
                    nc.vector.tensor_copy(self.v[b][sc][:], psv[:])
            return go

        base = self.starts[b] if deadlines else None
        kcb_ = kcb

        def dl(t):
            return base + t if deadlines else None

        for hf in range(2):
            for q2 in range(2):
                self.push_fill(qgroup(hf, q2), dl((hf * 2 + q2) * kcb_))
        for hf in range((nk + 1023) // 1024):
            n = min(1024, nk - hf * 1024)
            for k2 in range((n + 511) // 512):
                self.push_fill(kgroup(hf, k2, min(512, n - k2 * 512)),
                               dl(hf * 8 + k2 * 4))
            chunks = list(range(n // 128))
            for j in range(0, len(chunks), 2):
                self.push_fill(vgroup(hf, chunks[j:j + 2]),
                               dl(hf * 8 + chunks[j]))

    def push_o(self, b, qb):
        nc, ap = self.nc, self.ap

        def ochunk(sc):
            def go():
                ot = self.wrk.tile([128, 2, 512], F16, tag="ot", bufs=4,
                                   name="ot")
                for ih in range(2):
                    tg = "pqk" if ih == 0 else "pv"
                    po = self.psum.tile([128, 512], F32, tag=tg, bufs=1,
                                        name="po")
                    nc.tensor.matmul(
                        po[:], self.ctx[b][:, sc * 128:(sc + 1) * 128],
                        self.wo[:, ih * 512:(ih + 1) * 512],
                        start=True, stop=True)
                    nc.vector.tensor_copy(ot[:, ih, :], po[:])
                nc.sync.dma_start(
                    ap["out"][b, sc * 128:(sc + 1) * 128, :], ot[:])
            return go

        for sc in range(4 * qb, 4 * qb + 4):
            self.push_fill(ochunk(sc))

    def push_fill(self, fn, deadline=None):
        import heapq
        heapq.heappush(self.fills,
                       (10 ** 9 if deadline is None else deadline,
                        self._seq, fn))
        self._seq += 1

    def pop_fill(self):
        import heapq
        if self.fills:
            heapq.heappop(self.fills)[2]()

    def pop_due(self, tick):
        import heapq
        while self.fills and self.fills[0][0] <= tick:
            heapq.heappop(self.fills)[2]()

    def drain_fills(self):
        while self.fills:
            self.pop_fill()

    # ---- the global attention tile stream ----

    def run(self, order):
        nc = self.nc
        self.starts = {}
        t0 = 0
        for b in order:
            self.starts[b] = t0
            t0 += 4 * self.kcs[b]
        # first batch's projections ride the deadline schedule so attention
        # starts as soon as the first q/k/v slices are projected
        self.push_proj(order[0], deadlines=True)
        pushed = {order[0]}

        pend = None  # (avden, ex, b, kc, first, last, qb)

        def flush():
            nonlocal pend
            if pend is None:
                return
            avden, ex, b, kc, first, last, qb = pend
            pend = None
            ones = self.ones[:, 1 + b, :] if last else self.ones[:, 0, :]
            for h in range(2):
                nc.tensor.matmul(
                    avden[64 * h:64 * h + 64, 0, :],
                    self.v[b][kc][:, h, :], ex[:, h, :],
                    start=first, stop=last)
            for h in range(2):
                nc.tensor.matmul(
                    avden[64 * h:64 * h + 64, 1, :], ones,
                    ex[:, h, :], start=first, stop=last)
            if last:
                recb = self.wrk.tile([128, 512], F32, tag="recb", name="recb")
                nc.vector.reciprocal_approx_fast(recb[:], avden[:, 1, :])
                nc.vector.tensor_mul(
                    self.ctx[b][:, qb * 512:(qb + 1) * 512],
                    avden[:, 0, :], recb[:])
                self.push_o(b, qb)

        tick = 0
        for i, b in enumerate(order):
            kcb = self.kcs[b]
            # Prefetch projections for upcoming batches: the next one, plus
            # one more if the next attention segment is too small to host it.
            j = i + 1
            while j < len(order) and order[j] not in pushed:
                self.push_proj(order[j], deadlines=True)
                pushed.add(order[j])
                if self.kcs[order[j]] > 2:
                    break
                j += 1
            for qb in range(4):
                avden_cur = self.psum.tile([128, 2, 512], F32, tag="avden",
                                           bufs=1, name="avden")
                for kc in range(kcb):
                    self.pop_due(tick)
                    scp = self.psum.tile([128, 2, 512], F32, tag="sc",
                                         bufs=2, name="scp")
                    for h in range(2):
                        nc.tensor.matmul(
                            scp[:, h, :],
                            self.kT[b][64 * h:64 * h + 64,
                                       kc * 128:(kc + 1) * 128],
                            self.qT[b][64 * h:64 * h + 64,
                                       qb * 512:(qb + 1) * 512],
                            start=True, stop=True)
                    ex = self.expool.tile([128, 2, 512], BF16, tag="ex",
                                          name="ex")
                    nc.scalar.activation(ex[:], scp[:], AF.Exp, scale=0.125)
                    flush()
                    pend = (avden_cur, ex, b, kc, kc == 0, kc == kcb - 1, qb)
                    self.pop_fill()
                    if kc == kcb - 1:
                        # qb boundary: the single avden slot drains through
                        # recip+mul; feed the PE extra fill work meanwhile.
                        self.pop_fill()
                        self.pop_fill()
                    tick += 1
        flush()
        self.drain_fills()
        self.es.close()


def _emit(nc, tc, ap, kcs):
    em = _Emitter(nc, tc, ap, kcs)
    # Largest first (gets the startup shadow), smallest hosted mid-stream,
    # a large batch last so its own O-projection chunks are hidden.
    order = sorted(range(B), key=lambda b: -kcs[b])
    order = order[:-2] + [order[-1], order[-2]]
    em.run(order)


def _build(kcs):
    key = ("nc", tuple(kcs))
    if key in _cache:
        return _cache[key]
    nc = bacc.Bacc("TRN2", target_bir_lowering=False, debug=False,
                   num_devices=8)
    ap = {"wq": nc.dram_tensor("wq", [128, 8, 128], BF16,
                               kind="ExternalInput").ap(),
          "wk": nc.dram_tensor("wk", [128, 8, 128], BF16,
                               kind="ExternalInput").ap(),
          "wv": nc.dram_tensor("wv", [128, 8, 2, HD], BF16,
                               kind="ExternalInput").ap(),
          "wo": nc.dram_tensor("wo", [128, D], BF16,
                               kind="ExternalInput").ap(),
          "ones": nc.dram_tensor("ones", [128, 5, 64], BF16,
                                 kind="ExternalInput").ap(),
          "out": nc.dram_tensor("out", [B, S, D], F16,
                                kind="ExternalOutput").ap()}
    for b in range(B):
        ap[f"xq{b}"] = nc.dram_tensor(f"xq{b}", [128, 8, S], BF16,
                                      kind="ExternalInput").ap()
        ap[f"xk{b}"] = nc.dram_tensor(f"xk{b}", [128, 8, kcs[b] * 128], BF16,
                                      kind="ExternalInput").ap()
        ap[f"xv{b}"] = nc.dram_tensor(f"xv{b}", [128, 8, kcs[b] * 128], BF16,
                                      kind="ExternalInput").ap()
    with tile.TileContext(nc) as tc:
        _emit(nc, tc, ap, kcs)
    nc.compile()
    _cache[key] = nc
    return nc


def _blocked(x2d):
    """[Dsub, N] -> [128, Dsub//128, N] blocked layout (partition, dj, col)."""
    d, n = x2d.shape
    return np.ascontiguousarray(
        x2d.reshape(d // 128, 128, n).transpose(1, 0, 2))


def _in_maps(kcs, queries, keys, values, valid_len, W_q, W_k, W_v, W_o):
    bf = ml_dtypes.bfloat16
    shared = {}
    for b in range(B):
        nk = kcs[b] * 128
        xv = values[b][:nk].T.copy()      # [D, nk]
        xv[:, int(valid_len[b]):] = 0.0   # mask padding rows of v
        shared[f"xq{b}"] = _blocked(queries[b].T.astype(bf))
        shared[f"xk{b}"] = _blocked(keys[b][:nk].T.astype(bf))
        shared[f"xv{b}"] = _blocked(xv.astype(bf))
    ones = np.zeros((128, 5, 64), bf)
    ones[:, 0, :] = 1.0
    p = np.arange(128)
    for b in range(B):
        valid = ((kcs[b] - 1) * 128 + p < int(valid_len[b])).astype(bf)
        ones[:, 1 + b, :] = valid[:, None]
    shared["ones"] = ones

    maps = []
    for c in range(8):
        j0 = 128 * c
        m = dict(shared)
        m["wq"] = _blocked(
            np.ascontiguousarray(W_q[j0:j0 + 128, :].T).astype(bf))
        m["wk"] = _blocked(
            np.ascontiguousarray(W_k[j0:j0 + 128, :].T).astype(bf))
        m["wv"] = _blocked(
            np.ascontiguousarray(W_v[j0:j0 + 128, :].T).astype(bf)
        ).reshape(128, 8, 2, HD)
        m["wo"] = np.ascontiguousarray(W_o[:, j0:j0 + 128].T).astype(bf)
        maps.append(m)
    return maps


def kernel(queries, keys, values, valid_len, W_q, W_k, W_v, W_o,
           _run_kwargs=None):
    queries = np.asarray(queries, np.float32)
    keys = np.asarray(keys, np.float32)
    values = np.asarray(values, np.float32)
    valid_len = np.asarray(valid_len)
    W_q = np.asarray(W_q, np.float32)
    W_k = np.asarray(W_k, np.float32)
    W_v = np.asarray(W_v, np.float32)
    W_o = np.asarray(W_o, np.float32)

    kcs = [max(1, min(16, -(-int(valid_len[b]) // 128))) for b in range(B)]
    nc = _build(kcs)
    maps = _in_maps(kcs, queries, keys, values, valid_len, W_q, W_k, W_v, W_o)
    res = run_bass_kernel_spmd(nc, maps, list(range(8)), **(_run_kwargs or {}))
    out = np.zeros((B, S, D), np.float32)
    for c in range(8):
        out += res.results[c]["out"].astype(np.float32)
    if _run_kwargs:
        _cache["last_results"] = res
    return out


# revision 20
# speedup vs baseline: 1.0356x; 1.0356x over previous
"""Multi-head attention (B=4, S=2048, D=1024, H=16) on 8 Trainium2 cores.

Sharding: core c -> head-pair p = c (2 heads, 128 output dims), all 4
batches.  Every core runs the same per-batch k-loop trip counts
kc_b = ceil(valid_len[b]/128), so the key-padding truncation is SPMD-uniform.
W_o is row-split by head-pair; each core emits a full-shape [B, S, D] fp16
partial and the host sums the 8 partials.

v7: engines execute their instruction queues in emission order, so overlap
must be programmed, not hoped for.  The kernel is one global software-
pipelined stream over attention tiles (b, qb, kc):

    scores(t) -> exp(t) -> [av/den(t-1)] -> [norm when qb done] -> fill

where "fill" pops one closure from a queue holding Q/K/V-projection matmul
groups of the next batch and O-projection chunks of finished query blocks.
This keeps the PE busy during every exp wait and keeps ACT back-to-back.

Math per tile: scores = row-tiled concurrent matmul pair (K=64 at row
groups 0/64); AV = col-tiled concurrent pair (M=64 at col groups 0/64);
denominators via a second col-tiled pair with masked-ones lhsT, landing
den on the same partitions as the AV dims -> normalization is one
lane-aligned reciprocal_approx_fast + tensor_mul from PSUM.

PSUM: sc 2x2 banks + avden 2 + pqk 1 + pv 1 = 8 banks.
"""

import contextlib

import numpy as np
import ml_dtypes

import concourse.bacc as bacc
import concourse.mybir as mybir
import concourse.tile as tile
from concourse.bass_utils import run_bass_kernel_spmd

BF16 = mybir.dt.bfloat16
F16 = mybir.dt.float16
F32 = mybir.dt.float32
AF = mybir.ActivationFunctionType

B, S, D, H, HD = 4, 2048, 1024, 16, 64

_cache = {}


class _Emitter:
    def __init__(self, nc, tc, ap, kcs):
        self.nc = nc
        self.ap = ap
        self.kcs = kcs
        self.fills = []
        self._seq = 0

        es = self.es = contextlib.ExitStack()
        const = es.enter_context(tc.tile_pool(name="const", bufs=1))
        resid = es.enter_context(tc.tile_pool(name="resid", bufs=1))
        self.stream = es.enter_context(tc.tile_pool(name="stream", bufs=2))
        self.expool = es.enter_context(tc.tile_pool(name="expool", bufs=3))
        self.wrk = es.enter_context(tc.tile_pool(name="wrk", bufs=2))
        self.psum = es.enter_context(
            tc.tile_pool(name="psum", bufs=1, space="PSUM"))

        self.wq = const.tile([128, 8, 128], BF16, tag="wq", name="wq")
        self.wk = const.tile([128, 8, 128], BF16, tag="wk", name="wk")
        self.wv = const.tile([128, 8, 2, HD], BF16, tag="wv", name="wv")
        self.wo = const.tile([128, D], BF16, tag="wo", name="wo")
        self.ones = const.tile([128, 5, 64], BF16, tag="ones", name="ones")
        self.ident = const.tile([128, 128], BF16, tag="ident", name="ident")
        for n, t in [("wq", self.wq), ("wk", self.wk), ("wv", self.wv),
                     ("wo", self.wo), ("ones", self.ones),
                     ("ident", self.ident)]:
            nc.sync.dma_start(t[:], ap[n])

        self.qT = [resid.tile([128, S], BF16, tag=f"qT{b}", name=f"qT{b}")
                   for b in range(B)]
        self.kT = [resid.tile([128, kcs[b] * 128], BF16, tag=f"kT{b}",
                              name=f"kT{b}") for b in range(B)]
        self.ctx = [resid.tile([128, S], BF16, tag=f"ctx{b}", name=f"ctx{b}")
                    for b in range(B)]
        self.v = [[resid.tile([128, 128], BF16, tag=f"v{b}_{i}",
                              name=f"v{b}_{i}") for i in range(kcs[b])]
                  for b in range(B)]

    # ---- fill closures (projections of a later batch, O-proj chunks) ----

    def push_proj(self, b, deadlines=False):
        """Emit input DMAs for batch b now; queue its matmul groups.

        With deadlines=True (first batch), each closure carries the index of
        the earliest attention tile of batch b that needs its output, so the
        tile loop can force-pop them just in time and start attention while
        the rest of the projections interleave."""
        nc, ap, kcb = self.nc, self.ap, self.kcs[b]
        nk = kcb * 128
        xqt, xkt, xvt = {}, {}, {}
        for hf in range(2):
            t = self.stream.tile([128, 8, 1024], BF16, tag="xq", bufs=3,
                                 name="xqt")
            nc.sync.dma_start(t[:], ap[f"xq{b}"][:, :, hf * 1024:(hf + 1) * 1024])
            xqt[hf] = t
        for hf in range((nk + 1023) // 1024):
            n = min(1024, nk - hf * 1024)
            t = self.stream.tile([128, 8, 1024], BF16, tag="xk", name="xkt")
            nc.sync.dma_start(t[:, :, 0:n],
                              ap[f"xk{b}"][:, :, hf * 1024:hf * 1024 + n])
            xkt[hf] = t
            t = self.stream.tile([128, 8, 1024], BF16, tag="xv", name="xvt")
            nc.sync.dma_start(t[:, :, 0:n],
                              ap[f"xv{b}"][:, :, hf * 1024:hf * 1024 + n])
            xvt[hf] = t

        def qgroup(hf, q2):
            def go():
                qb = hf * 2 + q2
                psq = self.psum.tile([128, 512], F32, tag="pqk", bufs=1,
                                     name="psq")
                for dj in range(8):
                    nc.tensor.matmul(psq[:], self.wq[:, dj, :],
                                     xqt[hf][:, dj, q2 * 512:(q2 + 1) * 512],
                                     start=(dj == 0), stop=(dj == 7))
                nc.vector.tensor_copy(
                    self.qT[b][:, qb * 512:(qb + 1) * 512], psq[:])
            return go

        def kgroup(hf, k2, m):
            def go():
                psk = self.psum.tile([128, 512], F32, tag="pqk", bufs=1,
                                     name="psk")
                for dj in range(8):
                    nc.tensor.matmul(psk[:, 0:m], self.wk[:, dj, :],
                                     xkt[hf][:, dj, k2 * 512:k2 * 512 + m],
                                     start=(dj == 0), stop=(dj == 7))
                o = hf * 1024 + k2 * 512
                nc.vector.tensor_copy(self.kT[b][:, o:o + m], psk[:, 0:m])
            return go

        def vgroup(hf, blk, m):
            # vT [dout, keys] via N=512 matmuls (full-rate GEMM), then a PE
            # transpose per 128-key chunk to the [keys, dout] layout AV wants.
            def go():
                vT = self.psum.tile([128, 512], F32, tag="pqk", bufs=1,
                                    name="vTp")
                for dj in range(8):
                    nc.tensor.matmul(
                        vT[:, 0:m], self.wv[:, dj, :, :],
                        xvt[hf][:, dj, blk * 512:blk * 512 + m],
                        start=(dj == 0), stop=(dj == 7))
                vTs = self.wrk.tile([128, 512], BF16, tag="vTs", bufs=2,
                                    name="vTs")
                nc.vector.tensor_copy(vTs[:, 0:m], vT[:, 0:m])
                for t in range(m // 128):
                    sc = hf * 8 + blk * 4 + t
                    tp = self.psum.tile([128, 128], BF16, tag="pv", bufs=1,
                                        name="tp")
                    nc.tensor.transpose(tp[:], vTs[:, t * 128:(t + 1) * 128],
                                        self.ident[:])
                    nc.vector.tensor_copy(self.v[b][sc][:], tp[:])
            return go

        base = self.starts[b] if deadlines else None
        kcb_ = kcb

        def dl(t):
            return base + t if deadlines else None

        for hf in range(2):
            for q2 in range(2):
                self.push_fill(qgroup(hf, q2), dl((hf * 2 + q2) * kcb_))
        for hf in range((nk + 1023) // 1024):
            n = min(1024, nk - hf * 1024)
            for k2 in range((n + 511) // 512):
                self.push_fill(kgroup(hf, k2, min(512, n - k2 * 512)),
                               dl(hf * 8 + k2 * 4))
            for blk in range((n + 511) // 512):
                self.push_fill(vgroup(hf, blk, min(512, n - blk * 512)),
                               dl(hf * 8 + blk * 4))

    def push_o(self, b, qb):
        nc, ap = self.nc, self.ap

        def ochunk(sc):
            def go():
                ot = self.wrk.tile([128, 2, 512], F16, tag="ot", bufs=4,
                                   name="ot")
                for ih in range(2):
                    tg = "pqk" if ih == 0 else "pv"
                    po = self.psum.tile([128, 512], F32, tag=tg, bufs=1,
                                        name="po")
                    nc.tensor.matmul(
                        po[:], self.ctx[b][:, sc * 128:(sc + 1) * 128],
                        self.wo[:, ih * 512:(ih + 1) * 512],
                        start=True, stop=True)
                    nc.vector.tensor_copy(ot[:, ih, :], po[:])
                nc.sync.dma_start(
                    ap["out"][b, sc * 128:(sc + 1) * 128, :], ot[:])
            return go

        for sc in range(4 * qb, 4 * qb + 4):
            self.push_fill(ochunk(sc))

    def push_fill(self, fn, deadline=None):
        import heapq
        heapq.heappush(self.fills,
                       (10 ** 9 if deadline is None else deadline,
                        self._seq, fn))
        self._seq += 1

    def pop_fill(self):
        import heapq
        if self.fills:
            heapq.heappop(self.fills)[2]()

    def pop_due(self, tick):
        import heapq
        while self.fills and self.fills[0][0] <= tick:
            heapq.heappop(self.fills)[2]()

    def drain_fills(self):
        while self.fills:
            self.pop_fill()

    # ---- the global attention tile stream ----

    def run(self, order):
        nc = self.nc
        self.starts = {}
        t0 = 0
        for b in order:
            self.starts[b] = t0
            t0 += 4 * self.kcs[b]
        # startup: first batch's projections emitted as a block
        self.push_proj(order[0])
        self.drain_fills()
        pushed = {order[0]}

        pend = None  # (avden, ex, b, kc, first, last, qb)

        def flush():
            nonlocal pend
            if pend is None:
                return
            avden, ex, b, kc, first, last, qb = pend
            pend = None
            ones = self.ones[:, 1 + b, :] if last else self.ones[:, 0, :]
            for h in range(2):
                nc.tensor.matmul(
                    avden[64 * h:64 * h + 64, 0, :],
                    self.v[b][kc][:, 64 * h:64 * h + 64], ex[:, h, :],
                    start=first, stop=last)
            for h in range(2):
                nc.tensor.matmul(
                    avden[64 * h:64 * h + 64, 1, :], ones,
                    ex[:, h, :], start=first, stop=last)
            if last:
                recb = self.wrk.tile([128, 512], F32, tag="recb", name="recb")
                nc.vector.reciprocal_approx_fast(recb[:], avden[:, 1, :])
                nc.vector.tensor_mul(
                    self.ctx[b][:, qb * 512:(qb + 1) * 512],
                    avden[:, 0, :], recb[:])
                self.push_o(b, qb)

        tick = 0
        for i, b in enumerate(order):
            kcb = self.kcs[b]
            # Prefetch projections for upcoming batches: the next one, plus
            # one more if the next attention segment is too small to host it.
            j = i + 1
            while j < len(order) and order[j] not in pushed:
                self.push_proj(order[j])
                pushed.add(order[j])
                if self.kcs[order[j]] > 2:
                    break
                j += 1
            for qb in range(4):
                avden_cur = self.psum.tile([128, 2, 512], F32, tag="avden",
                                           bufs=1, name="avden")
                for kc in range(kcb):
                    self.pop_due(tick)
                    scp = self.psum.tile([128, 2, 512], F32, tag="sc",
                                         bufs=2, name="scp")
                    for h in range(2):
                        nc.tensor.matmul(
                            scp[:, h, :],
                            self.kT[b][64 * h:64 * h + 64,
                                       kc * 128:(kc + 1) * 128],
                            self.qT[b][64 * h:64 * h + 64,
                                       qb * 512:(qb + 1) * 512],
                            start=True, stop=True)
                    ex = self.expool.tile([128, 2, 512], BF16, tag="ex",
                                          name="ex")
                    nc.scalar.activation(ex[:], scp[:], AF.Exp, scale=0.125)
                    flush()
                    pend = (avden_cur, ex, b, kc, kc == 0, kc == kcb - 1, qb)
                    self.pop_fill()
                    tick += 1
        flush()
        self.drain_fills()
        self.es.close()


def _emit(nc, tc, ap, kcs):
    em = _Emitter(nc, tc, ap, kcs)
    # Largest first (gets the startup shadow), smallest hosted mid-stream,
    # a large batch last so its own O-projection chunks are hidden.
    order = sorted(range(B), key=lambda b: -kcs[b])
    order = order[:-2] + [order[-1], order[-2]]
    em.run(order)


def _build(kcs):
    key = ("nc", tuple(kcs))
    if key in _cache:
        return _cache[key]
    nc = bacc.Bacc("TRN2", target_bir_lowering=False, debug=False,
                   num_devices=8)
    ap = {"wq": nc.dram_tensor("wq", [128, 8, 128], BF16,
                               kind="ExternalInput").ap(),
          "wk": nc.dram_tensor("wk", [128, 8, 128], BF16,
                               kind="ExternalInput").ap(),
          "wv": nc.dram_tensor("wv", [128, 8, 2, HD], BF16,
                               kind="ExternalInput").ap(),
          "wo": nc.dram_tensor("wo", [128, D], BF16,
                               kind="ExternalInput").ap(),
          "ones": nc.dram_tensor("ones", [128, 5, 64], BF16,
                                 kind="ExternalInput").ap(),
          "ident": nc.dram_tensor("ident", [128, 128], BF16,
                                  kind="ExternalInput").ap(),
          "out": nc.dram_tensor("out", [B, S, D], F16,
                                kind="ExternalOutput").ap()}
    for b in range(B):
        ap[f"xq{b}"] = nc.dram_tensor(f"xq{b}", [128, 8, S], BF16,
                                      kind="ExternalInput").ap()
        ap[f"xk{b}"] = nc.dram_tensor(f"xk{b}", [128, 8, kcs[b] * 128], BF16,
                                      kind="ExternalInput").ap()
        ap[f"xv{b}"] = nc.dram_tensor(f"xv{b}", [128, 8, kcs[b] * 128], BF16,
                                      kind="ExternalInput").ap()
    with tile.TileContext(nc) as tc:
        _emit(nc, tc, ap, kcs)
    nc.compile()
    _cache[key] = nc
    return nc


def _blocked(x2d):
    """[Dsub, N] -> [128, Dsub//128, N] blocked layout (partition, dj, col)."""
    d, n = x2d.shape
    return np.ascontiguousarray(
        x2d.reshape(d // 128, 128, n).transpose(1, 0, 2))


def _in_maps(kcs, queries, keys, values, valid_len, W_q, W_k, W_v, W_o):
    bf = ml_dtypes.bfloat16
    shared = {}
    for b in range(B):
        nk = kcs[b] * 128
        xv = values[b][:nk].T.copy()      # [D, nk]
        xv[:, int(valid_len[b]):] = 0.0   # mask padding rows of v
        shared[f"xq{b}"] = _blocked(queries[b].T.astype(bf))
        shared[f"xk{b}"] = _blocked(keys[b][:nk].T.astype(bf))
        shared[f"xv{b}"] = _blocked(xv.astype(bf))
    ones = np.zeros((128, 5, 64), bf)
    ones[:, 0, :] = 1.0
    p = np.arange(128)
    for b in range(B):
        valid = ((kcs[b] - 1) * 128 + p < int(valid_len[b])).astype(bf)
        ones[:, 1 + b, :] = valid[:, None]
    shared["ones"] = ones
    shared["ident"] = np.eye(128, dtype=bf)

    maps = []
    for c in range(8):
        j0 = 128 * c
        m = dict(shared)
        m["wq"] = _blocked(
            np.ascontiguousarray(W_q[j0:j0 + 128, :].T).astype(bf))
        m["wk"] = _blocked(
            np.ascontiguousarray(W_k[j0:j0 + 128, :].T).astype(bf))
        m["wv"] = _blocked(
            np.ascontiguousarray(W_v[j0:j0 + 128, :].T).astype(bf)
        ).reshape(128, 8, 2, HD)
        m["wo"] = np.ascontiguousarray(W_o[:, j0:j0 + 128].T).astype(bf)
        maps.append(m)
    return maps


def kernel(queries, keys, values, valid_len, W_q, W_k, W_v, W_o,
           _run_kwargs=None):
    queries = np.asarray(queries, np.float32)
    keys = np.asarray(keys, np.float32)
    values = np.asarray(values, np.float32)
    valid_len = np.asarray(valid_len)
    W_q = np.asarray(W_q, np.float32)
    W_k = np.asarray(W_k, np.float32)
    W_v = np.asarray(W_v, np.float32)
    W_o = np.asarray(W_o, np.float32)

    kcs = [max(1, min(16, -(-int(valid_len[b]) // 128))) for b in range(B)]
    nc = _build(kcs)
    maps = _in_maps(kcs, queries, keys, values, valid_len, W_q, W_k, W_v, W_o)
    res = run_bass_kernel_spmd(nc, maps, list(range(8)), **(_run_kwargs or {}))
    out = np.zeros((B, S, D), np.float32)
    for c in range(8):
        out += res.results[c]["out"].astype(np.float32)
    if _run_kwargs:
        _cache["last_results"] = res
    return out
